# revision 31
# baseline (speedup 1.0000x reference)
import sys
sys.path.insert(0, '/opt/trn_rl_repo')

"""Multi-head attention TP kernel for TRN2 — per-core program builder.

Sharding: 8 cores = 2 (batch) x 4 (head groups of 4 heads = 512 dims).
Each core computes, for its batch b and head-dim slice e:
    q = x[b] @ wq[e,:].T + bq[e]      (stored transposed: qT [E, S])
    k = x[b] @ wk[e,:].T + bk[e]      (kT [E, S])
    v = x[b] @ wv[e,:].T + bv[e]      (v [S, E])
    per head h (dh=128): ST = K-major score tiles, exp (no max-sub; scores
    bounded ~|3|), AV accumulated unnormalized.  Softmax denominator: DVE
    bf16 add-tree over the exp tiles (pairs->quads->octs->one) + a single
    all-ones matmul for the partition sum; normalized on psO eviction with
    reciprocal_approx_fast.
    partial_out = attn_out @ wo[:, e].T   ([S, D]; host sums 8 partials + bo)

The kernel is PE-cycle-bound, so everything that is not a projection /
score / AV matmul is kept off the tensor engine: biases via DVE
tensor_scalar / gpsimd-broadcast adds, exp on the scalar engine in N=1024
batches (2 PSUM banks per ACTIVATE), denominator tree on DVE.  All PSUM
evictions run on DVE — a copy in the scalar FIFO would delay trailing
exps and stall the score pipeline.  The WO projection is interleaved into
the NEXT i-block's attention (one 4-matmul group per exp pair) to fill
the stalls where ACT lags the PE.  The projection phase shares the
attention PSUM pools so there is no pool-close barrier between phases.
x streams through a rolling 2-chunk buffer; DMAs are issued in k-group
interleaved order so the first matmul starts after ~1.6MB has landed.
Output is bf16 (host accumulates partials in fp32).
"""

import math

import numpy as np

import concourse.bass as bass
import concourse.tile as tile
from concourse import bacc, mybir

F32 = mybir.dt.float32
BF16 = mybir.dt.bfloat16
AF = mybir.ActivationFunctionType


def build_module(
    S=2048,          # sequence per core (one batch)
    D=2048,          # model dim
    E=512,           # head dims per core (4 heads x 128)
    bufs_es=6,
    enable_asserts=False,
):
    HD = 128
    SC = 512
    NK = D // HD        # proj contraction tiles
    NH = E // HD        # heads per core
    NSC = S // SC       # s-chunks / i-blocks
    NJ = S // HD        # attention j tiles
    NJG = NJ // 2       # j-tile pairs (exp batches)
    ND = D // SC        # WO n-chunks
    scale = 1.0 / math.sqrt(HD)

    nc = bacc.Bacc(
        "TRN2",
        target_bir_lowering=False,
        debug=False,
        enable_asserts=enable_asserts,
        num_devices=8,
    )

    # x is s-chunk-major: [HD, NSC * NK * SC]
    xr = nc.dram_tensor("xr", [HD, NSC * NK * SC], BF16,
                        kind="ExternalInput").ap()
    wqt = nc.dram_tensor("wqt", [HD, NK * E], BF16, kind="ExternalInput").ap()
    wkt = nc.dram_tensor("wkt", [HD, NK * E], BF16, kind="ExternalInput").ap()
    wvt = nc.dram_tensor("wvt", [HD, NK * E], BF16, kind="ExternalInput").ap()
    wot = nc.dram_tensor("wot", [HD, NH * D], BF16, kind="ExternalInput").ap()
    bqc = nc.dram_tensor("bqc", [HD, NH], F32, kind="ExternalInput").ap()
    bkc = nc.dram_tensor("bkc", [HD, NH], F32, kind="ExternalInput").ap()
    bvr = nc.dram_tensor("bvr", [1, E], BF16, kind="ExternalInput").ap()
    ones2d = nc.dram_tensor("ones2d", [HD, HD], BF16,
                            kind="ExternalInput").ap()
    out = nc.dram_tensor("out", [S, D], BF16, kind="ExternalOutput").ap()

    with tile.TileContext(nc) as tc:
        with (
            tc.tile_pool(name="qkv", bufs=1) as qkv_pool,
            tc.tile_pool(name="consts", bufs=1) as consts,
            tc.tile_pool(name="wbuf", bufs=1) as w_pool,
            tc.tile_pool(name="xroll", bufs=2) as x_pool,
        ):
            q_sb = qkv_pool.tile([HD, NH, S], BF16)
            k_sb = qkv_pool.tile([HD, NH, S], BF16)
            v_sb = qkv_pool.tile([HD, NJ, E], BF16)

            wq_sb = w_pool.tile([HD, NK, E], BF16, tag="wq")
            wk_sb = w_pool.tile([HD, NK, E], BF16, tag="wk")
            wv_sb = w_pool.tile([HD, NK, E], BF16, tag="wv")
            wo_sb = w_pool.tile([HD, NH, D], BF16, tag="wo")

            bq_sb = consts.tile([HD, NH], F32)
            bk_sb = consts.tile([HD, NH], F32)
            bv_sb = consts.tile([1, E], BF16)
            bv_bc = consts.tile([HD, E], BF16)
            allones = consts.tile([HD, HD], BF16)    # bcast-sum stationary

            # ---- DMA issue order = need order.  One HWDGE trigger queue
            # drains FIFO, so interleave per k-group: the kk=0..3 matmuls
            # of s-chunk 0 only need the first ~1.6MB.
            NG = NK // 4

            def load_w_k(dst, src, width, k0, nk=4):
                nc.sync.dma_start(
                    out=dst[:, k0:k0 + nk, :],
                    in_=src[:, k0 * width:(k0 + nk) * width]
                    .rearrange("p (k e) -> p k e", e=width))

            x_tiles = [None] * NSC

            def load_x_k(si, k0, nk=4):
                if x_tiles[si] is None:
                    x_tiles[si] = x_pool.tile([HD, NK, SC], BF16, tag="x",
                                              name=f"x{si}")
                off = si * NK * SC + k0 * SC
                nc.sync.dma_start(
                    out=x_tiles[si][:, k0:k0 + nk, :],
                    in_=xr[:, off:off + nk * SC].rearrange(
                        "p (k s) -> p k s", s=SC))

            def load_w_g(dst, src, width, g):
                load_w_k(dst, src, width, 4 * g)

            def load_x_g(si, g):
                load_x_k(si, 4 * g)

            for g in range(NG):
                load_w_g(wq_sb, wqt, E, g)
                load_w_g(wk_sb, wkt, E, g)
                load_x_g(0, g)
            nc.sync.dma_start(out=bq_sb, in_=bqc)
            nc.sync.dma_start(out=bk_sb, in_=bkc)
            nc.sync.dma_start(out=bv_sb, in_=bvr)
            nc.sync.dma_start(out=allones, in_=ones2d)
            for g in range(NG):
                load_w_g(wv_sb, wvt, E, g)
            for g in range(NG):
                load_x_g(1, g)
            nc.sync.dma_start(
                out=wo_sb, in_=wot.rearrange("p (k d) -> p k d", d=D))
            nc.gpsimd.partition_broadcast(bv_bc, bv_sb)

            # ------- Phases A + C/D share the PSUM pools so attention's
            # first scores can start while the last V-chunk still runs -----
            with (
                tc.tile_pool(name="outT", bufs=1) as outT_pool,
                tc.tile_pool(name="attws", bufs=2) as ws_pool,
                tc.tile_pool(name="es", bufs=bufs_es) as es_pool,
                tc.tile_pool(name="esp", bufs=3) as esp_pool,
                tc.tile_pool(name="esr", bufs=2) as esr_pool,
                tc.tile_pool(name="og", bufs=2) as og_pool,
                tc.tile_pool(name="psS", bufs=2, space="PSUM") as psS_pool,
                tc.tile_pool(name="psOW", bufs=3, space="PSUM") as psOW_pool,
                tc.tile_pool(name="psN", bufs=1, space="PSUM") as psN_pool,
            ):
                # ------ Phase A: fused Q,K,V projections per s-chunk ------
                # Q accumulates in the "s" tiles (2x 2-bank), K and V in
                # the "ow"/"bc" banks, so attention allocations chain onto
                # the projections with no pool-close barrier.
                for si in range(NSC):
                    s0 = si * SC
                    xt = x_tiles[si]
                    xv = xt.rearrange("p k (t h) -> p k t h", h=HD)
                    psQt = [psS_pool.tile([HD, 2, SC], F32, tag="s",
                                          name=f"psqt{p}") for p in range(2)]
                    psQ = [psQt[0][:, 0, :], psQt[0][:, 1, :],
                           psQt[1][:, 0, :], psQt[1][:, 1, :]]
                    psK = [psOW_pool.tile([HD, SC], F32, tag="ow",
                                          name=f"psk{m}") for m in range(3)]
                    psK.append(psN_pool.tile([HD, SC], F32, tag="bc",
                                             name="psk3"))
                    for kk in range(NK):
                        st = kk == 0
                        sp = kk == NK - 1
                        for m in range(NH):
                            nc.tensor.matmul(
                                psQ[m],
                                wq_sb[:, kk, m * HD:(m + 1) * HD],
                                xt[:, kk, :],
                                start=st, stop=sp,
                            )
                            nc.tensor.matmul(
                                psK[m],
                                wk_sb[:, kk, m * HD:(m + 1) * HD],
                                xt[:, kk, :],
                                start=st, stop=sp,
                            )
                    for m in range(NH):
                        nc.vector.tensor_scalar_add(
                            q_sb[:, m, s0:s0 + SC], psQ[m],
                            bq_sb[:, m:m + 1])
                        nc.vector.tensor_scalar_add(
                            k_sb[:, m, s0:s0 + SC], psK[m],
                            bk_sb[:, m:m + 1])
                    # prefetch x for si+2 now that chunk si is nearly done
                    if si + 2 < NSC:
                        for g in range(NG):
                            load_x_g(si + 2, g)
                    # V for this chunk: x stationary, wv moving
                    psV = [psOW_pool.tile([HD, E], F32, tag="ow",
                                          name=f"psv{mv}") for mv in range(3)]
                    psV.append(psN_pool.tile([HD, E], F32, tag="bc",
                                             name="psv3"))
                    for kk in range(NK):
                        for mv in range(NH):
                            nc.tensor.matmul(
                                psV[mv],
                                xv[:, kk, mv, :],
                                wv_sb[:, kk, :],
                                start=(kk == 0), stop=(kk == NK - 1),
                            )
                    for mv in range(NH):
                        nc.vector.tensor_add(
                            v_sb[:, si * NH + mv, :], psV[mv], bv_bc)

                outT_sb = outT_pool.tile([HD, NH, S], BF16)

                # WO work queue: groups of (it, nn) drained one per jg-pair
                # inside the NEXT i-block's attention, so WO matmuls fill
                # the PE stalls where ACT (exp) lags within a block.
                wo_pending = []
                og_map = {}

                def emit_wo_group():
                    if not wo_pending:
                        return
                    it, nn = wo_pending.pop(0)
                    if nn == 0:
                        og_map[it] = og_pool.tile([HD, D], BF16, tag="og",
                                                  name="og")
                    og = og_map[it]
                    psW = psOW_pool.tile([HD, SC], F32, tag="ow",
                                         name="psW")
                    for kk in range(NH):
                        nc.tensor.matmul(
                            psW,
                            outT_sb[:, kk, it * HD:(it + 1) * HD],
                            wo_sb[:, kk, nn * SC:(nn + 1) * SC],
                            start=(kk == 0), stop=(kk == NH - 1),
                        )
                    # keep og eviction off ACT: a copy in the scalar FIFO
                    # delays trailing exps, stalling the psS pipeline
                    nc.vector.tensor_copy(
                        og[:, nn * SC:(nn + 1) * SC], psW)
                    if nn == ND - 1:
                        nc.sync.dma_start(
                            out=out[it * HD:(it + 1) * HD, :], in_=og)
                        del og_map[it]

                for ib in range(NSC):
                    i0 = ib * SC
                    for h in range(NH):
                        psO = psOW_pool.tile([HD, SC], F32, tag="ow",
                                             name="psO")
                        ps_bc = psN_pool.tile([HD, SC], F32, tag="bc")
                        es_t = [None] * NJG

                        def emit_av(jg, h=h, psO=psO, es_t=es_t):
                            for u in range(2):
                                j = 2 * jg + u
                                nc.tensor.matmul(
                                    psO,
                                    v_sb[:, j, h * HD:(h + 1) * HD],
                                    es_t[jg][:, u, :],
                                    start=(j == 0), stop=(j == NJ - 1),
                                )

                        # DVE (bf16, 2x rate) reduction tree over the exp
                        # tiles: pairs -> quads -> octs -> one tile, which
                        # a single all-ones matmul turns into the softmax
                        # denominator (partition-broadcast in PSUM).
                        esp_prev = [None]
                        esq_prev = [None]
                        eso_prev = [None]
                        esf = [None]
                        for jg in range(NJG):
                            psS = psS_pool.tile([HD, 2, SC], F32, tag="s")
                            for u in range(2):
                                j = 2 * jg + u
                                nc.tensor.matmul(
                                    psS[:, u, :],
                                    k_sb[:, h, j * HD:(j + 1) * HD],
                                    q_sb[:, h, i0:i0 + SC],
                                    start=True, stop=True,
                                )
                            if jg >= 2:
                                emit_av(jg - 2)
                            if jg % 2 == 1:
                                emit_wo_group()
                            es = es_pool.tile([HD, 2, SC], BF16, tag="es",
                                              name="es")
                            nc.scalar.activation(es, psS, AF.Exp,
                                                 scale=scale)
                            es_t[jg] = es
                            # tree adds at N=1024 (whole es tiles) to
                            # minimize DVE instruction count: 4+2+1 wide
                            # adds + one 512-wide fold
                            if jg % 2 == 1:
                                esp = esp_pool.tile([HD, 2, SC], BF16,
                                                    tag="esp", name="esp")
                                nc.vector.tensor_add(esp, es_t[jg - 1], es)
                                if jg % 4 == 3:
                                    esq = esp_pool.tile([HD, 2, SC], BF16,
                                                        tag="esq",
                                                        name="esq")
                                    nc.vector.tensor_add(
                                        esq, esp_prev[0], esp)
                                    if jg == NJG - 1:
                                        eso = esr_pool.tile(
                                            [HD, 2, SC], BF16, tag="eso",
                                            name="eso")
                                        nc.vector.tensor_add(
                                            eso, esq_prev[0], esq)
                                        esf[0] = esr_pool.tile(
                                            [HD, SC], BF16, tag="esf",
                                            name="esf")
                                        nc.vector.tensor_add(
                                            esf[0], eso[:, 0, :],
                                            eso[:, 1, :])
                                    esq_prev[0] = esq
                                esp_prev[0] = esp
                        emit_av(NJG - 2)
                        emit_av(NJG - 1)
                        nc.tensor.matmul(ps_bc, allones, esf[0],
                                         start=True, stop=True)
                        recip = ws_pool.tile([HD, SC], F32, tag="recip")
                        nc.vector.reciprocal_approx_fast(recip, ps_bc)
                        nc.vector.tensor_mul(
                            outT_sb[:, h, i0:i0 + SC], psO, recip)

                    # queue this ib's WO groups; they drain inside the
                    # next i-block's attention (one per jg pair)
                    for t in range(NSC):
                        it = ib * NSC + t
                        for nn in range(ND):
                            wo_pending.append((it, nn))

                while wo_pending:
                    emit_wo_group()

    nc.compile()
    return nc


# ---------------------------------------------------------------------------
# Host-side sharding helpers
# ---------------------------------------------------------------------------

def _bf16(a):
    import ml_dtypes
    return np.asarray(a).astype(ml_dtypes.bfloat16)


def make_in_map(x_b, wq_e, bq_e, wk_e, bk_e, wv_e, bv_e, wo_e):
    """Per-core input dict. x_b [S, D]; w*_e [E, D] row slices; wo_e [D, E]
    column slice; b*_e [E]."""
    E = wq_e.shape[0]
    S, D = x_b.shape
    HD = 128
    SC = 512
    NH = E // HD
    NK = D // HD
    NSC = S // SC

    def wrelayout(wT):  # [D, E'] -> [HD, NK*E'] with k-tile-major columns
        Ew = wT.shape[1]
        return _bf16(
            wT.reshape(NK, HD, Ew).transpose(1, 0, 2).reshape(HD, NK * Ew))

    xT = x_b.T  # [D, S]
    # s-chunk-major x: xr[hd, si, k, s] = xT[k*HD+hd, si*SC+s]
    xr = xT.reshape(NK, HD, NSC, SC).transpose(1, 2, 0, 3).reshape(HD, -1)
    return {
        "xr": _bf16(xr),
        "wqt": wrelayout(wq_e.T),
        "wkt": wrelayout(wk_e.T),
        "wvt": wrelayout(wv_e.T),
        "wot": _bf16(
            wo_e.T.reshape(NH, HD, D).transpose(1, 0, 2).reshape(HD, NH * D)),
        "bqc": np.ascontiguousarray(bq_e.reshape(NH, HD).T),
        "bkc": np.ascontiguousarray(bk_e.reshape(NH, HD).T),
        "bvr": _bf16(bv_e.reshape(1, E)),
        "ones2d": _bf16(np.ones((HD, HD), np.float32)),
    }


def core_reference(x_b, wq_e, bq_e, wk_e, bk_e, wv_e, bv_e, wo_e):
    """Numpy reference for one core's partial output."""
    HD = 128
    q = x_b @ wq_e.T + bq_e
    k = x_b @ wk_e.T + bk_e
    v = x_b @ wv_e.T + bv_e
    E = q.shape[1]
    outs = []
    for h in range(E // HD):
        qh = q[:, h * HD:(h + 1) * HD]
        kh = k[:, h * HD:(h + 1) * HD]
        vh = v[:, h * HD:(h + 1) * HD]
        s = (qh @ kh.T) / math.sqrt(HD)
        p = np.exp(s)
        outs.append((p @ vh) / p.sum(-1, keepdims=True))
    o = np.concatenate(outs, axis=1)
    return o @ wo_e.T


# ---------------------------------------------------------------------------
# Entry point: full-input kernel with internal 8-way sharding
# ---------------------------------------------------------------------------

import os as _os

_NC_CACHE = {}


def _get_module():
    if "nc" not in _NC_CACHE:
        _NC_CACHE["nc"] = build_module(S=2048, D=2048, E=512)
    return _NC_CACHE["nc"]


def kernel(x, wq, bq, wk, bk, wv, bv, wo, bo):
    """Full inputs -> full output. 8 cores = 2 (batch) x 4 (head-group)."""
    from concourse import bass_utils

    x = np.asarray(x, dtype=np.float32)
    wq, bq = np.asarray(wq, np.float32), np.asarray(bq, np.float32)
    wk, bk = np.asarray(wk, np.float32), np.asarray(bk, np.float32)
    wv, bv = np.asarray(wv, np.float32), np.asarray(bv, np.float32)
    wo, bo = np.asarray(wo, np.float32), np.asarray(bo, np.float32)

    E = 512
    nc = _get_module()
    in_maps = []
    for c in range(8):
        b, g = divmod(c, 4)
        e = slice(g * E, (g + 1) * E)
        in_maps.append(make_in_map(
            x[b], wq[e], bq[e], wk[e], bk[e], wv[e], bv[e], wo[:, e]))

    trace = bool(int(_os.environ.get("ATTN_TRACE", "0")))
    kw = {}
    if trace:
        tmpdir = _os.environ.get("ATTN_TRACE_DIR") or None
        kw = dict(trace=True, tmpdir=tmpdir, trace_cores=[0])
    res = bass_utils.run_bass_kernel_spmd(
        nc, in_maps, core_ids=list(range(8)), **kw)
    if trace:
        print(f"HW exec time: {res.exec_time_ns} ns")
        _NC_CACHE["last_results"] = res

    y = np.empty((2, 2048, 2048), np.float32)
    for b in range(2):
        acc = np.asarray(res.results[4 * b]["out"], np.float32)
        for g in range(1, 4):
            acc += np.asarray(res.results[4 * b + g]["out"], np.float32)
        y[b] = acc + bo
    return y
